# revision 1
# baseline (speedup 1.0000x reference)
"""HandGSConsistencyLoss on 8 NeuronCores.

Data-parallel over the B*S=64 frame axis: each of the 8 cores holds 8 frames
(mask, gs_means, hand_params, hand_valid) plus the replicated MANO tables,
computes its own topk/gather/cdist partial (weighted sum, weight count), and
the scalar pair is all-reduced across cores for the final mean.
"""
import numpy as np
import jax
import jax.numpy as jnp

HAND_PARAM_DIM = 32
MASK_THRESHOLD = 0.5
MAX_SAMPLES = 1024
INF_DIST = 1.0e6
EPS = 1e-8
B, S, HW, V = 4, 16, 65536, 778
N_CORES = 8
F = (B * S) // N_CORES  # frames per core


def _quat_wxyz_to_rotvec(q):
    q_norm = jnp.sqrt(jnp.sum(q * q, axis=-1, keepdims=True) + EPS)
    q = q / jnp.maximum(q_norm, EPS)
    w = q[..., 0:1]
    xyz = q[..., 1:4]
    sign = jnp.where(w < 0, -1.0, 1.0)
    w = w * sign
    xyz = xyz * sign
    sin_half = jnp.sqrt(jnp.sum(xyz * xyz, axis=-1, keepdims=True) + EPS)
    w_safe = jnp.clip(w, -1.0 + EPS, 1.0 - EPS)
    angle = 2.0 * jnp.arctan2(sin_half, w_safe)
    factor = jnp.where(sin_half < 1e-6, 2.0, angle / jnp.maximum(sin_half, EPS))
    return xyz * factor


def _rodrigues(pts, rotvec):
    angle = jnp.sqrt(jnp.sum(rotvec * rotvec, axis=-1, keepdims=True) + EPS)
    axis = rotvec / angle
    c = jnp.cos(angle)[:, None, :]
    s = jnp.sin(angle)[:, None, :]
    k = axis[:, None, :]
    kxv = jnp.cross(jnp.broadcast_to(k, pts.shape), pts)
    kdv = jnp.sum(k * pts, axis=-1, keepdims=True)
    return pts * c + kxv * s + k * kdv * (1.0 - c)


def _mano_points(p, v_template, shapedirs, posedirs):
    transl = p[:, :3]
    q = p[:, 3:7]
    hand_pose = p[:, 7:22]
    betas = p[:, 22:32]
    rotvec = _quat_wxyz_to_rotvec(q)
    pts = (v_template[None]
           + jnp.einsum('vck,nk->nvc', shapedirs, betas)
           + jnp.einsum('vck,nk->nvc', posedirs, hand_pose))
    return _rodrigues(pts, rotvec) + transl[:, None, :]


def _shard_loss(hand_params, gs_means, hand_gs_mask, hand_valid,
                vtl, vtr, sdl, sdr, pdl, pdr):
    # hand_params [F,2,32], gs_means [F,HW,3], hand_gs_mask [F,HW], hand_valid [F,2]
    hv = hand_valid.astype(bool)
    left = _mano_points(hand_params[:, 0], vtl, sdl, pdl)
    right = _mano_points(hand_params[:, 1], vtr, sdr, pdr)
    left = jnp.where(hv[:, 0:1, None], left, INF_DIST)
    right = jnp.where(hv[:, 1:2, None], right, INF_DIST)
    mesh = jnp.concatenate([left, right], axis=1)           # [F, 2V, 3]

    top_vals, top_idx = jax.lax.top_k(hand_gs_mask, MAX_SAMPLES)  # [F, K]
    selv = (top_vals > MASK_THRESHOLD) & jnp.any(hv, axis=-1, keepdims=True)
    sampled = jnp.take_along_axis(gs_means, top_idx[..., None], axis=1)  # [F,K,3]

    sa = jnp.sum(sampled * sampled, axis=-1)
    sb = jnp.sum(mesh * mesh, axis=-1)
    d2 = jnp.maximum(sa[:, :, None] + sb[:, None, :]
                     - 2.0 * jnp.einsum('nkc,npc->nkp', sampled, mesh), 0.0)
    nn_d2 = jnp.min(d2, axis=-1)                            # [F, K]

    w = selv.astype(nn_d2.dtype)
    num = jax.lax.psum(jnp.sum(nn_d2 * w), 'i')             # all-reduce scalar pair
    den = jax.lax.psum(jnp.sum(w), 'i')
    return num / jnp.maximum(den, 1.0)


def _host_fallback(hp, gm, mk, hv_, vtl, vtr, sdl, sdr, pdl, pdr):
    # Pure-numpy replica (used only if the device path fails).
    def np_rotvec(q):
        qn = np.sqrt((q * q).sum(-1, keepdims=True) + EPS)
        q = q / np.maximum(qn, EPS)
        w, xyz = q[..., 0:1], q[..., 1:4]
        sgn = np.where(w < 0, -1.0, 1.0)
        w, xyz = w * sgn, xyz * sgn
        sh = np.sqrt((xyz * xyz).sum(-1, keepdims=True) + EPS)
        ws = np.clip(w, -1 + EPS, 1 - EPS)
        ang = 2.0 * np.arctan2(sh, ws)
        fac = np.where(sh < 1e-6, 2.0, ang / np.maximum(sh, EPS))
        return (xyz * fac).astype(np.float32)

    def np_mano(p, vt, sd, pd):
        tr, q, hp_, be = p[:, :3], p[:, 3:7], p[:, 7:22], p[:, 22:32]
        rv = np_rotvec(q)
        pts = vt[None] + np.einsum('vck,nk->nvc', sd, be) + np.einsum('vck,nk->nvc', pd, hp_)
        ang = np.sqrt((rv * rv).sum(-1, keepdims=True) + EPS)
        ax = rv / ang
        c = np.cos(ang)[:, None, :]; s = np.sin(ang)[:, None, :]
        k = ax[:, None, :]
        kxv = np.cross(np.broadcast_to(k, pts.shape), pts)
        kdv = (k * pts).sum(-1, keepdims=True)
        return (pts * c + kxv * s + k * kdv * (1 - c) + tr[:, None, :]).astype(np.float32)

    hvb = hv_.astype(bool)
    left = np_mano(hp[:, 0], vtl, sdl, pdl)
    right = np_mano(hp[:, 1], vtr, sdr, pdr)
    left = np.where(hvb[:, 0:1, None], left, INF_DIST).astype(np.float32)
    right = np.where(hvb[:, 1:2, None], right, INF_DIST).astype(np.float32)
    mesh = np.concatenate([left, right], 1)
    num = 0.0; den = 0.0
    for f in range(hp.shape[0]):
        if not hvb[f].any():
            continue
        idx = np.argpartition(mk[f], HW - MAX_SAMPLES)[HW - MAX_SAMPLES:]
        vals = mk[f][idx]
        keep = vals > MASK_THRESHOLD
        s_ = gm[f][idx[keep]]
        sa = (s_ * s_).sum(-1); sb = (mesh[f] * mesh[f]).sum(-1)
        d2 = np.maximum(sa[:, None] + sb[None, :] - 2.0 * s_ @ mesh[f].T, 0.0)
        num += d2.min(-1).sum(); den += keep.sum()
    return np.float32(num / max(den, 1.0))


def kernel(**inputs):
    hp = np.asarray(inputs["hand_params"], dtype=np.float32).reshape(B * S, 2, HAND_PARAM_DIM)
    gm = np.asarray(inputs["gs_means"], dtype=np.float32).reshape(B * S, HW, 3)
    mk = np.asarray(inputs["hand_gs_mask"], dtype=np.float32).reshape(B * S, HW)
    hv = np.asarray(inputs["hand_valid"]).reshape(B * S, 2)
    reps = [np.asarray(inputs[k], dtype=np.float32) for k in
            ("v_template_l", "v_template_r", "shapedirs_l", "shapedirs_r",
             "posedirs_l", "posedirs_r")]

    def shard(x):
        return x.reshape(N_CORES, F, *x.shape[1:])

    try:
        fn = jax.pmap(_shard_loss, axis_name='i',
                      in_axes=(0, 0, 0, 0, None, None, None, None, None, None))
        out = fn(shard(hp), shard(gm), shard(mk), shard(hv), *reps)
        res = np.asarray(out)[0]
        if not np.isfinite(res):
            raise FloatingPointError("non-finite device result")
        return np.float32(res)
    except Exception:
        return _host_fallback(hp, gm, mk, hv, *reps)


if __name__ == "__main__":
    rng = np.random.default_rng(0)
    demo = {
        "hand_params": rng.standard_normal((B, S, 64)).astype(np.float32),
        "gs_means": rng.standard_normal((B, S, HW, 3)).astype(np.float32),
        "hand_gs_mask": rng.random((B, S, HW)).astype(np.float32),
        "hand_valid": rng.integers(0, 2, (B, S, 2)).astype(np.int32),
        "v_template_l": rng.standard_normal((V, 3)).astype(np.float32),
        "v_template_r": rng.standard_normal((V, 3)).astype(np.float32),
        "shapedirs_l": (rng.standard_normal((V, 3, 10)) * 0.1).astype(np.float32),
        "shapedirs_r": (rng.standard_normal((V, 3, 10)) * 0.1).astype(np.float32),
        "posedirs_l": (rng.standard_normal((V, 3, 15)) * 0.1).astype(np.float32),
        "posedirs_r": (rng.standard_normal((V, 3, 15)) * 0.1).astype(np.float32),
    }
    print(kernel(**demo))


# revision 3
# speedup vs baseline: 1.0206x; 1.0206x over previous
"""HandGSConsistencyLoss on 8 NeuronCores.

Data-parallel over the B*S=64 frame axis: each of the 8 cores holds 8 frames
(mask, gs_means, hand_params, hand_valid) plus the replicated MANO tables,
computes its own topk/gather/cdist partial (weighted sum, weight count), and
the scalar pair is all-reduced across cores for the final mean.
"""
import numpy as np
import jax
import jax.numpy as jnp

HAND_PARAM_DIM = 32
MASK_THRESHOLD = 0.5
MAX_SAMPLES = 1024
INF_DIST = 1.0e6
EPS = 1e-8
B, S, HW, V = 4, 16, 65536, 778
N_CORES = 8
F = (B * S) // N_CORES  # frames per core


def _quat_wxyz_to_rotvec(q):
    q_norm = jnp.sqrt(jnp.sum(q * q, axis=-1, keepdims=True) + EPS)
    q = q / jnp.maximum(q_norm, EPS)
    w = q[..., 0:1]
    xyz = q[..., 1:4]
    sign = jnp.where(w < 0, -1.0, 1.0)
    w = w * sign
    xyz = xyz * sign
    sin_half = jnp.sqrt(jnp.sum(xyz * xyz, axis=-1, keepdims=True) + EPS)
    w_safe = jnp.clip(w, -1.0 + EPS, 1.0 - EPS)
    angle = 2.0 * jnp.arctan2(sin_half, w_safe)
    factor = jnp.where(sin_half < 1e-6, 2.0, angle / jnp.maximum(sin_half, EPS))
    return xyz * factor


def _rodrigues(pts, rotvec):
    angle = jnp.sqrt(jnp.sum(rotvec * rotvec, axis=-1, keepdims=True) + EPS)
    axis = rotvec / angle
    c = jnp.cos(angle)[:, None, :]
    s = jnp.sin(angle)[:, None, :]
    k = axis[:, None, :]
    kxv = jnp.cross(jnp.broadcast_to(k, pts.shape), pts)
    kdv = jnp.sum(k * pts, axis=-1, keepdims=True)
    return pts * c + kxv * s + k * kdv * (1.0 - c)


def _mano_points(p, v_template, shapedirs, posedirs):
    transl = p[:, :3]
    q = p[:, 3:7]
    hand_pose = p[:, 7:22]
    betas = p[:, 22:32]
    rotvec = _quat_wxyz_to_rotvec(q)
    pts = (v_template[None]
           + jnp.einsum('vck,nk->nvc', shapedirs, betas)
           + jnp.einsum('vck,nk->nvc', posedirs, hand_pose))
    return _rodrigues(pts, rotvec) + transl[:, None, :]


def _shard_loss(hand_params, gs_means, hand_gs_mask, hand_valid,
                vtl, vtr, sdl, sdr, pdl, pdr):
    # hand_params [F,2,32], gs_means [F,HW,3], hand_gs_mask [F,HW], hand_valid [F,2]
    hv = hand_valid.astype(bool)
    left = _mano_points(hand_params[:, 0], vtl, sdl, pdl)
    right = _mano_points(hand_params[:, 1], vtr, sdr, pdr)
    left = jnp.where(hv[:, 0:1, None], left, INF_DIST)
    right = jnp.where(hv[:, 1:2, None], right, INF_DIST)
    mesh = jnp.concatenate([left, right], axis=1)           # [F, 2V, 3]

    top_vals, top_idx = jax.lax.top_k(hand_gs_mask, MAX_SAMPLES)  # [F, K]
    selv = (top_vals > MASK_THRESHOLD) & jnp.any(hv, axis=-1, keepdims=True)
    sampled = jnp.take_along_axis(gs_means, top_idx[..., None], axis=1)  # [F,K,3]

    sa = jnp.sum(sampled * sampled, axis=-1)
    sb = jnp.sum(mesh * mesh, axis=-1)
    d2 = jnp.maximum(sa[:, :, None] + sb[:, None, :]
                     - 2.0 * jnp.einsum('nkc,npc->nkp', sampled, mesh), 0.0)
    nn_d2 = jnp.min(d2, axis=-1)                            # [F, K]

    w = selv.astype(nn_d2.dtype)
    num = jax.lax.psum(jnp.sum(nn_d2 * w), 'i')             # all-reduce scalar pair
    den = jax.lax.psum(jnp.sum(w), 'i')
    return num / jnp.maximum(den, 1.0)


def _host_fallback(hp, gm, mk, hv_, vtl, vtr, sdl, sdr, pdl, pdr):
    # Pure-numpy replica (used only if the device path fails).
    def np_rotvec(q):
        qn = np.sqrt((q * q).sum(-1, keepdims=True) + EPS)
        q = q / np.maximum(qn, EPS)
        w, xyz = q[..., 0:1], q[..., 1:4]
        sgn = np.where(w < 0, -1.0, 1.0)
        w, xyz = w * sgn, xyz * sgn
        sh = np.sqrt((xyz * xyz).sum(-1, keepdims=True) + EPS)
        ws = np.clip(w, -1 + EPS, 1 - EPS)
        ang = 2.0 * np.arctan2(sh, ws)
        fac = np.where(sh < 1e-6, 2.0, ang / np.maximum(sh, EPS))
        return (xyz * fac).astype(np.float32)

    def np_mano(p, vt, sd, pd):
        tr, q, hp_, be = p[:, :3], p[:, 3:7], p[:, 7:22], p[:, 22:32]
        rv = np_rotvec(q)
        pts = vt[None] + np.einsum('vck,nk->nvc', sd, be) + np.einsum('vck,nk->nvc', pd, hp_)
        ang = np.sqrt((rv * rv).sum(-1, keepdims=True) + EPS)
        ax = rv / ang
        c = np.cos(ang)[:, None, :]; s = np.sin(ang)[:, None, :]
        k = ax[:, None, :]
        kxv = np.cross(np.broadcast_to(k, pts.shape), pts)
        kdv = (k * pts).sum(-1, keepdims=True)
        return (pts * c + kxv * s + k * kdv * (1 - c) + tr[:, None, :]).astype(np.float32)

    hvb = hv_.astype(bool)
    left = np_mano(hp[:, 0], vtl, sdl, pdl)
    right = np_mano(hp[:, 1], vtr, sdr, pdr)
    left = np.where(hvb[:, 0:1, None], left, INF_DIST).astype(np.float32)
    right = np.where(hvb[:, 1:2, None], right, INF_DIST).astype(np.float32)
    mesh = np.concatenate([left, right], 1)
    num = 0.0; den = 0.0
    for f in range(hp.shape[0]):
        if not hvb[f].any():
            continue
        idx = np.argpartition(mk[f], HW - MAX_SAMPLES)[HW - MAX_SAMPLES:]
        vals = mk[f][idx]
        keep = vals > MASK_THRESHOLD
        s_ = gm[f][idx[keep]]
        sa = (s_ * s_).sum(-1); sb = (mesh[f] * mesh[f]).sum(-1)
        d2 = np.maximum(sa[:, None] + sb[None, :] - 2.0 * s_ @ mesh[f].T, 0.0)
        num += d2.min(-1).sum(); den += keep.sum()
    return np.float32(num / max(den, 1.0))


_PMAP_CACHE = []


def _get_pmap_fn():
    if not _PMAP_CACHE:
        _PMAP_CACHE.append(jax.pmap(
            _shard_loss, axis_name='i',
            in_axes=(0, 0, 0, 0, None, None, None, None, None, None)))
    return _PMAP_CACHE[0]


def kernel(**inputs):
    hp = np.asarray(inputs["hand_params"], dtype=np.float32).reshape(B * S, 2, HAND_PARAM_DIM)
    gm = np.asarray(inputs["gs_means"], dtype=np.float32).reshape(B * S, HW, 3)
    mk = np.asarray(inputs["hand_gs_mask"], dtype=np.float32).reshape(B * S, HW)
    hv = np.asarray(inputs["hand_valid"]).reshape(B * S, 2)
    reps = [np.asarray(inputs[k], dtype=np.float32) for k in
            ("v_template_l", "v_template_r", "shapedirs_l", "shapedirs_r",
             "posedirs_l", "posedirs_r")]

    def shard(x):
        return x.reshape(N_CORES, F, *x.shape[1:])

    try:
        fn = _get_pmap_fn()
        out = fn(shard(hp), shard(gm), shard(mk), shard(hv), *reps)
        res = np.asarray(out)[0]
        if not np.isfinite(res):
            raise FloatingPointError("non-finite device result")
        return np.float32(res)
    except Exception:
        return _host_fallback(hp, gm, mk, hv, *reps)


if __name__ == "__main__":
    rng = np.random.default_rng(0)
    demo = {
        "hand_params": rng.standard_normal((B, S, 64)).astype(np.float32),
        "gs_means": rng.standard_normal((B, S, HW, 3)).astype(np.float32),
        "hand_gs_mask": rng.random((B, S, HW)).astype(np.float32),
        "hand_valid": rng.integers(0, 2, (B, S, 2)).astype(np.int32),
        "v_template_l": rng.standard_normal((V, 3)).astype(np.float32),
        "v_template_r": rng.standard_normal((V, 3)).astype(np.float32),
        "shapedirs_l": (rng.standard_normal((V, 3, 10)) * 0.1).astype(np.float32),
        "shapedirs_r": (rng.standard_normal((V, 3, 10)) * 0.1).astype(np.float32),
        "posedirs_l": (rng.standard_normal((V, 3, 15)) * 0.1).astype(np.float32),
        "posedirs_r": (rng.standard_normal((V, 3, 15)) * 0.1).astype(np.float32),
    }
    print(kernel(**demo))
